# revision 2
# baseline (speedup 1.0000x reference)
"""Multi-head causal self-attention with RoPE on 8 Trainium2 cores (v2).

Reference semantics (d_model=1024, 16 heads, d_h=64, rope theta 1e4):
    qkv = x @ W_qkv.T ; q,k = rope(q),rope(k)
    out = softmax(causal(q k^T / 8)) @ v ; return out @ W_out.T

Sharding: core c -> (batch b = c//2, head-group hg = c%2, 8 heads each).
Each core computes a partial output projection for its head group; the
host sums the two partials per batch. No on-device collectives.

v2 vs baseline:
  - bf16 operands everywhere (fp32 PSUM accumulation, fp32 output);
    halves DMA traffic and SBUF footprint, enables 2x DVE modes.
  - V stays resident in SBUF (no DRAM spill/reload).
  - PSUM evacuation copies, softmax normalize (partition_broadcast +
    divide) run on the idle GpSimd/Pool engine (proxy ucode library).
  - diagonal score tiles compute only the live column subrange
    (scores matmul, exp, PV all sliced; single [128,128] tri mask).
  - software-pipelined emission: attention for token-block nt (Act-heavy)
    is interleaved instruction-by-instruction with the projections for
    block nt+1 and the output projection of block nt-1 (PE-heavy), so
    the tensor engine never drains while exp runs.
"""

import numpy as np
import ml_dtypes

BF16 = ml_dtypes.bfloat16

D_MODEL = 1024
SEQ = 2048
N_HEADS = 16
D_H = 64
H_PER_CORE = 8
ROPE_THETA = 10000.0
N_CORES = 8

TQ = 512          # token block (free-dim tile)
NT = SEQ // TQ    # 4
NDC = D_MODEL // 128  # 8 contraction chunks


# ---------------------------------------------------------------- host math

def _a_perm():
    """A-layout row order for one 512-row head group (8 heads x 64 dims).

    chunk0: even dims of heads 0-3, chunk1: even dims of heads 4-7,
    chunk2: odd dims of heads 0-3,  chunk3: odd dims of heads 4-7.
    """
    idx = []
    for parity in (0, 1):
        for group in (0, 1):
            for h in range(4):
                for f in range(32):
                    idx.append((group * 4 + h) * 64 + 2 * f + parity)
    return np.array(idx, dtype=np.int64)


def _perm_mats():
    """[P_e0, P_e1, P_o0, P_o1] as [src, dst] 0/1 matrices.

    HC chunk c (heads 2c, 2c+1; rows [h: evens(32) odds(32)]) is
    P_e(c%2).T @ A_even(c//2) + P_o(c%2).T @ A_odd(c//2).
    """
    mats = np.zeros((4, 128, 128), np.float32)
    for cm in range(2):
        for d in range(128):
            hp, within = d // 64, d % 64
            parity, f = within // 32, within % 32
            s = (2 * cm + hp) * 32 + f
            mats[parity * 2 + cm, s, d] = 1.0
    return mats


def prep_core_inputs(x, token_positions, W_qkv, W_out, core):
    b, hg = core // 2, core % 2
    ap = _a_perm()

    Wq = W_qkv[hg * 512:(hg + 1) * 512]
    Wk = W_qkv[D_MODEL + hg * 512:D_MODEL + (hg + 1) * 512]
    Wv = W_qkv[2 * D_MODEL + hg * 512:2 * D_MODEL + (hg + 1) * 512]

    pos = token_positions.astype(np.float32)
    invf = 1.0 / (ROPE_THETA ** (np.arange(0, D_H, 2, dtype=np.float32) / D_H))
    ang = pos[None, :] * invf[np.arange(128) % 32, None]      # [128, SEQ]

    tri = (np.arange(128)[:, None] <= np.arange(128)[None, :])

    return {
        "xT": np.ascontiguousarray(x[b].T).astype(BF16),
        "wqkT": np.ascontiguousarray(
            np.concatenate([Wq[ap], Wk[ap]], axis=0).T).astype(BF16),
        "wvT": np.ascontiguousarray(Wv.T).astype(BF16),
        "woutT": np.ascontiguousarray(
            W_out[:, hg * 512:(hg + 1) * 512].T).astype(BF16),
        "cosA": np.ascontiguousarray(np.cos(ang)),
        "sinA": np.ascontiguousarray(np.sin(ang)),
        "tri": tri.astype(BF16),
        "perm": _perm_mats().astype(BF16),
        "ones": np.kron(np.eye(2, dtype=np.float32),
                        np.ones((1, 64), np.float32)),
    }


# ---------------------------------------------------------------- bass build

def build_bass(split_waits=True):
    import concourse.bass as bass
    import concourse.mybir as mybir
    import concourse.tile as tile

    f32 = mybir.dt.float32
    bf16 = mybir.dt.bfloat16

    nc = bass.Bass("TRN2", target_bir_lowering=False, debug=False)
    # this walrus build cannot encode the raw-ISA RANGE_CLEAR emitted by
    # gpsimd.sem_clear in the kernel tail; NRT re-initializes semaphores per
    # execution, so replace it with a nop (verified by repeat-run checks).
    nc.gpsimd.sem_clear = lambda rng: nc.gpsimd.nop(hint="semclear_skip")

    class S:
        pass

    s = S()
    s.nc = nc
    s.f32, s.bf16 = f32, bf16
    s.f32r = mybir.dt.float32r
    s.EXP = mybir.ActivationFunctionType.Exp

    s.xT = nc.declare_dram_parameter("xT", [D_MODEL, SEQ], bf16, isOutput=False)
    s.wqkT = nc.declare_dram_parameter("wqkT", [D_MODEL, 1024], bf16, isOutput=False)
    s.wvT = nc.declare_dram_parameter("wvT", [D_MODEL, 512], bf16, isOutput=False)
    s.woutT = nc.declare_dram_parameter("woutT", [512, D_MODEL], bf16, isOutput=False)
    s.cosA = nc.declare_dram_parameter("cosA", [128, SEQ], f32, isOutput=False)
    s.sinA = nc.declare_dram_parameter("sinA", [128, SEQ], f32, isOutput=False)
    s.triP = nc.declare_dram_parameter("tri", [128, 128], bf16, isOutput=False)
    s.permP = nc.declare_dram_parameter("perm", [4, 128, 128], bf16, isOutput=False)
    s.onesP = nc.declare_dram_parameter(
        "ones", [2, 128], mybir.dt.float32r, isOutput=False)
    s.out = nc.declare_dram_parameter("out", [SEQ, D_MODEL], f32, isOutput=True)

    with tile.TileContext(nc) as tc:
        s.tc = tc
        with (
            tc.tile_pool(name="persist", bufs=1) as s.p_per,
            tc.tile_pool(name="rt", bufs=4) as s.p_rt,
            tc.tile_pool(name="re", bufs=4) as s.p_re,
            tc.tile_pool(name="e", bufs=8) as s.p_e,
            tc.tile_pool(name="rep", bufs=2) as s.p_rep,
            tc.tile_pool(name="ob", bufs=2) as s.p_ob,
            tc.tile_pool(name="psA", bufs=2, space="PSUM") as s.ps_A,
            tc.tile_pool(name="psS", bufs=3, space="PSUM") as s.ps_S,
            tc.tile_pool(name="psU", bufs=2, space="PSUM") as s.ps_U,
            tc.tile_pool(name="psM", bufs=1, space="PSUM") as s.ps_M,
        ):
            _emit(s)

    if split_waits:
        _split_sync_waits(nc)
    return nc


def _split_sync_waits(nc, limit=1):
    """walrus in this container rejects instructions with more than ~1 sync
    wait. Move excess waits onto preceding same-engine NOPs (engine streams
    execute in order, so the waits still complete before the instruction)."""
    import concourse.mybir as mybir
    n = 0
    for fn in nc.m.functions:
        for blk in fn.blocks:
            out = []
            for inst in blk.instructions:
                si = inst.sync_info
                waits = list(si.on_wait) if si is not None else []
                if len(waits) > limit:
                    for w in waits[:-limit]:
                        n += 1
                        nop = mybir.InstNoOp(
                            name=f"wsplit-{n}",
                            engine=inst.engine,
                            sync_info=mybir.SyncInfo(on_wait=[w], on_update=[]),
                        )
                        out.append(nop)
                    inst.sync_info = mybir.SyncInfo(
                        on_wait=waits[-limit:], on_update=list(si.on_update))
                out.append(inst)
            blk.instructions = out
    return n


# ------------------------------------------------------------- emission

def _alloc_persistent(s):
    p, bf16, f32 = s.p_per, s.bf16, s.f32
    # x^T chunks, per (dim-chunk, token-block)
    s.xt = [[p.tile([128, TQ], bf16, tag=f"xt{kc}_{b}", name=f"xt{kc}_{b}")
             for b in range(NT)] for kc in range(NDC)]
    # weights
    s.wqk = [p.tile([128, 1024], bf16, tag=f"wqk{kc}", name=f"wqk{kc}")
             for kc in range(NDC)]
    s.wv = [p.tile([128, 512], bf16, tag=f"wv{kc}", name=f"wv{kc}")
            for kc in range(NDC)]
    s.wo = [p.tile([128, 1024], bf16, tag=f"wo{kc}", name=f"wo{kc}")
            for kc in range(4)]
    # rope tables per block
    s.cos = [p.tile([128, TQ], f32, tag=f"cos{b}", name=f"cos{b}")
             for b in range(NT)]
    s.sin = [p.tile([128, TQ], f32, tag=f"sin{b}", name=f"sin{b}")
             for b in range(NT)]
    s.tri = p.tile([128, 128], bf16, tag="tri", name="tri")
    s.ones2 = p.tile([2, 128], s.f32r, tag="ones", name="ones")
    s.perm = [p.tile([128, 128], bf16, tag=f"pm{j}", name=f"pm{j}")
              for j in range(4)]
    # q/k head-contiguous chunks, per (chunk, token-block)
    s.q = [[p.tile([128, TQ], bf16, tag=f"q{c}_{b}", name=f"q{c}_{b}")
            for b in range(NT)] for c in range(4)]
    s.k = [[p.tile([128, TQ], bf16, tag=f"k{c}_{b}", name=f"k{c}_{b}")
            for b in range(NT)] for c in range(4)]
    # V with per-head ones column, per 128-token tile
    s.v = [p.tile([128, H_PER_CORE * 65], bf16, tag=f"v{t}", name=f"v{t}")
           for t in range(SEQ // 128)]
    # attention output chunks, per (chunk, token-block)
    s.ao = [[p.tile([128, TQ], bf16, tag=f"ao{c}_{b}", name=f"ao{c}_{b}")
             for b in range(NT)] for c in range(4)]


def _dma_block(s, b):
    """Prefetch x columns + rope tables for token block b."""
    nc = s.nc
    sl = slice(b * TQ, (b + 1) * TQ)
    for kc in range(NDC):
        nc.sync.dma_start(s.xt[kc][b][:], s.xT[kc * 128:(kc + 1) * 128, sl])
    nc.sync.dma_start(s.cos[b][:], s.cosA[:, sl])
    nc.sync.dma_start(s.sin[b][:], s.sinA[:, sl])


def _v_unit(s, b, tt):
    """V projection for 128-token tile tt (inside block b); yields per step."""
    nc = s.nc
    t0 = (tt % 4) * 128
    vp = s.ps_M.tile([128, 512], s.f32, tag="mps", name="vps")
    for kc in range(NDC):
        nc.tensor.matmul(
            vp[:], s.xt[kc][b][:, t0:t0 + 128], s.wv[kc][:],
            start=(kc == 0), stop=(kc == NDC - 1))
        yield
    nc.vector.tensor_copy(
        s.v[tt][:].rearrange("p (h d) -> p h d", d=65)[:, :, 0:64],
        vp[:].rearrange("p (h d) -> p h d", d=64))
    yield


def _qk_unit(s, half, pair, b):
    """Project A-chunk pair for one block, rope, permute to HC layout."""
    nc, f32, bf16 = s.nc, s.f32, s.bf16
    ce = half * 4 + pair
    co = half * 4 + 2 + pair
    pe = s.ps_A.tile([128, TQ], f32, tag="aps", name="pe")
    po = s.ps_A.tile([128, TQ], f32, tag="aps", name="po")
    for kc in range(NDC):
        nc.tensor.matmul(
            pe[:], s.wqk[kc][:, ce * 128:(ce + 1) * 128], s.xt[kc][b][:],
            start=(kc == 0), stop=(kc == NDC - 1))
        yield
    for kc in range(NDC):
        nc.tensor.matmul(
            po[:], s.wqk[kc][:, co * 128:(co + 1) * 128], s.xt[kc][b][:],
            start=(kc == 0), stop=(kc == NDC - 1))
        yield
    # rope: e' = e*cos - o*sin ; o' = e*sin + o*cos   (ops ordered so each
    # psum tile is released after its two reads)
    a = s.p_rt.tile([128, TQ], f32, tag="rt", name="rt_a")
    a2 = s.p_rt.tile([128, TQ], f32, tag="rt", name="rt_a2")
    bb = s.p_rt.tile([128, TQ], f32, tag="rt", name="rt_b")
    b2 = s.p_rt.tile([128, TQ], f32, tag="rt", name="rt_b2")
    re = s.p_re.tile([128, TQ], bf16, tag="re", name="re")
    ro = s.p_re.tile([128, TQ], bf16, tag="re", name="ro")
    nc.vector.tensor_mul(a[:], pe[:], s.cos[b][:])
    nc.vector.tensor_mul(a2[:], pe[:], s.sin[b][:])
    yield
    nc.vector.tensor_mul(bb[:], po[:], s.sin[b][:])
    nc.vector.tensor_mul(b2[:], po[:], s.cos[b][:])
    yield
    nc.vector.tensor_sub(re[:], a[:], bb[:])
    nc.vector.tensor_add(ro[:], a2[:], b2[:])
    yield
    hc_tiles = s.q if half == 0 else s.k
    for cc in (0, 1):
        c = 2 * pair + cc
        pp = s.ps_A.tile([128, TQ], f32, tag="aps", name="pp")
        nc.tensor.matmul(pp[:], s.perm[cc][:], re[:], start=True, stop=False)
        yield
        nc.tensor.matmul(pp[:], s.perm[2 + cc][:], ro[:], start=False, stop=True)
        yield
        nc.vector.tensor_copy(hc_tiles[c][b][:], pp[:])
        yield


def _out_unit(s, b, mt):
    """Output projection for 128-token tile mt of block b."""
    nc = s.nc
    t0 = (mt % 4) * 128
    ob = s.p_ob.tile([128, D_MODEL], s.f32, tag="ob", name="ob")
    for do in range(2):
        op = s.ps_M.tile([128, 512], s.f32, tag="mps", name="ops")
        for kc in range(4):
            nc.tensor.matmul(
                op[:], s.ao[kc][b][:, t0:t0 + 128],
                s.wo[kc][:, do * 512:(do + 1) * 512],
                start=(kc == 0), stop=(kc == 3))
            yield
        nc.vector.tensor_copy(ob[:, do * 512:(do + 1) * 512], op[:])
        yield
    nc.sync.dma_start(s.out[mt * 128:(mt + 1) * 128, :], ob[:])
    yield


def _attn_s(s, h, qt, kt):
    """Scores matmul for one (head, q-block, k-tile); exp comes a step later
    so the Activation engine never stalls on a just-issued matmul."""
    nc = s.nc
    hc, ro = h // 2, (h % 2) * 64
    j = kt - 4 * qt          # >= 0 on the 4 diagonal tiles
    lo = max(j, 0) * 128     # first live column within the q block
    kb, k0 = kt // 4, (kt % 4) * 128
    sp = s.ps_S.tile([128, TQ], s.f32, tag="sps", name="sps")
    nc.tensor.matmul(
        sp[:, lo:TQ],
        s.k[hc][kb][ro:ro + 64, k0:k0 + 128],
        s.q[hc][qt][ro:ro + 64, lo:TQ],
        start=True, stop=True)
    return (h, kt, lo, j, sp)


def _attn_exp(s, item):
    h, kt, lo, j, sp = item
    e = s.p_e.tile([128, TQ], s.bf16, tag="e", name="e")
    s.nc.scalar.activation(e[:, lo:TQ], sp[:, lo:TQ], s.EXP, scale=0.125)
    if j >= 0:
        s.nc.gpsimd.tensor_mul(
            e[:, lo:lo + 128], e[:, lo:lo + 128], s.tri[:])
    return (h, kt, lo, j, e)


def _attn_pv(s, qt, item):
    """PV accumulation; triggers the pair normalize when a head retires."""
    h, kt, lo, j, e = item
    if kt == 0:
        s.u_ps[h] = s.ps_U.tile([65, TQ], s.f32, tag="ups", name=f"u{h % 2}")
    s.nc.tensor.matmul(
        s.u_ps[h][:, lo:TQ],
        s.v[kt][:, h * 65:(h + 1) * 65],
        e[:, lo:TQ],
        start=(kt == 0), stop=(j == 3))
    if j == 3:
        _attn_norm(s, h, qt)


def _attn_norm(s, h, qt):
    """ao[head dims] = U[0:64] * broadcast(1 / U[64]): DVE reciprocal
    (PSUM row -> SBUF), K=1 ones matmul broadcasts across partitions, DVE
    copy, DVE multiply.  (GPSIMD cannot touch PSUM on hw and this walrus
    build cannot encode the PartitionBroadcast ISA op.)"""
    nc = s.nc
    hc, ro = h // 2, (h % 2) * 64
    rec = s.p_rep.tile([1, TQ], s.f32r, tag="rec", name="rec")
    with nc.allow_low_precision(reason="fp32r is fp32-width"):
        nc.vector.reciprocal(rec[:], s.u_ps[h][64:65, :])
    rpp = s.ps_M.tile([64, TQ], s.f32, tag="mps", name="rpp")
    nc.tensor.matmul(rpp[:], s.ones2[0:1, 0:64], rec[:], start=True, stop=True)
    rep = s.p_rep.tile([64, TQ], s.f32, tag="rep", name="rep")
    nc.vector.tensor_copy(rep[:], rpp[:])
    nc.vector.tensor_mul(
        s.ao[hc][qt][ro:ro + 64, :], s.u_ps[h][0:64, :], rep[:])


def _emit(s):
    nc = s.nc
    _alloc_persistent(s)

    # ---- prologue
    # interleave x-block0 and wv chunks so the first V-proj matmuls can
    # start as soon as their own operands land (DMA engine is serial)
    sl0 = slice(0, TQ)
    for kc in range(NDC):
        nc.sync.dma_start(s.xt[kc][0][:], s.xT[kc * 128:(kc + 1) * 128, sl0])
        nc.sync.dma_start(s.wv[kc][:], s.wvT[kc * 128:(kc + 1) * 128, :])
    nc.sync.dma_start(s.cos[0][:], s.cosA[:, sl0])
    nc.sync.dma_start(s.sin[0][:], s.sinA[:, sl0])
    for kc in range(NDC):
        nc.sync.dma_start(s.wqk[kc][:], s.wqkT[kc * 128:(kc + 1) * 128, :])
    nc.sync.dma_start(s.tri[:], s.triP[:])
    nc.sync.dma_start(s.ones2[:], s.onesP[:])
    for j in range(4):
        nc.sync.dma_start(s.perm[j][:], s.permP[j])
    _dma_block(s, 1)
    for kc in range(4):
        nc.sync.dma_start(s.wo[kc][:], s.woutT[kc * 128:(kc + 1) * 128, :])
    for t in range(SEQ // 128):
        nc.gpsimd.memset(
            s.v[t][:].rearrange("p (h d) -> p h d", d=65)[:, :, 64:65], 1.0)

    # block-0 projections (nothing to overlap with yet)
    for tt in range(4):
        for _ in _v_unit(s, 0, tt):
            pass
    for pair in range(2):
        for half in (0, 1):
            for _ in _qk_unit(s, half, pair, 0):
                pass

    # ---- steady-state pipeline over q blocks
    for qt in range(NT):
        if qt + 2 < NT:
            _dma_block(s, qt + 2)

        NV, NQK, NOUT = 9, 25, 11    # micro-steps per unit type
        units, n_micro = [], 0
        if qt == 0:
            units += [_v_unit(s, 1, tt) for tt in range(4, 8)]
            units += [_qk_unit(s, half, pair, 1)
                      for pair in range(2) for half in (0, 1)]
            n_micro = 4 * NV + 4 * NQK
        elif qt == 1:
            units += [_out_unit(s, 0, mt) for mt in range(0, 4)]
            units += [_v_unit(s, 2, tt) for tt in range(8, 12)]
            units += [_qk_unit(s, half, pair, 2)
                      for pair in range(2) for half in (0, 1)]
            n_micro = 4 * NOUT + 4 * NV + 4 * NQK
        elif qt == 2:
            units += [_out_unit(s, 1, mt) for mt in range(4, 8)]
            units += [_v_unit(s, 3, tt) for tt in range(12, 16)]
            n_micro = 4 * NOUT + 4 * NV
        else:
            units += [_out_unit(s, 2, mt) for mt in range(8, 12)]
            n_micro = 4 * NOUT

        # qt=3's q/k arrive per head-pair chunk, emitted just-in-time below
        late_qk = {}
        if qt == 3:
            late_qk = {0: [_qk_unit(s, 0, 0, 3), _qk_unit(s, 1, 0, 3)],
                       2: [_qk_unit(s, 0, 1, 3), _qk_unit(s, 1, 1, 3)]}

        def steps():
            for u in units:
                yield from u
        step_iter = steps()
        n_kt = 4 * (4 * qt + 4)      # kt steps across the 4 head pairs
        per_step, acc = n_micro / max(n_kt, 1), 0.0

        s.u_ps = {}
        s.norm_ready = []
        sq, pend = [], []

        def micro():
            nonlocal acc
            acc += per_step / 2.0
            while acc >= 1.0:
                acc -= 1.0
                try:
                    next(step_iter)
                except StopIteration:
                    acc = -1e9

        for hp in range(4):
            for u in late_qk.get(2 * hp, ()):   # just-in-time q/k for qt=3
                for _ in u:
                    pass
            h0, h1 = 2 * hp, 2 * hp + 1
            for kt in range(4 * qt + 4):
                while sq:                       # exps for the previous step
                    pend.append(_attn_exp(s, sq.pop(0)))
                sq.append(_attn_s(s, h0, qt, kt))
                if len(pend) > 2:
                    _attn_pv(s, qt, pend.pop(0))
                micro()
                sq.append(_attn_s(s, h1, qt, kt))
                if len(pend) > 2:
                    _attn_pv(s, qt, pend.pop(0))
                micro()
        while sq:                               # drain the pair pipeline
            pend.append(_attn_exp(s, sq.pop(0)))
        while pend:
            micro()
            _attn_pv(s, qt, pend.pop(0))
        for _ in step_iter:
            pass

    # ---- epilogue: last output projection
    for mt in range(12, 16):
        for _ in _out_unit(s, 3, mt):
            pass


# ---------------------------------------------------------------- execution

_CACHE = {}


def _get_runner():
    if "fn" in _CACHE:
        return _CACHE["fn"]
    import jax
    import numpy as _np
    from jax.sharding import Mesh, PartitionSpec
    from jax.experimental.shard_map import shard_map
    import concourse.mybir as mybir
    from concourse import bass2jax

    bass2jax.install_neuronx_cc_hook()
    nc = build_bass()

    partition_name = (
        nc.partition_id_tensor.name if nc.partition_id_tensor else None)
    in_names, out_names, out_avals, zero_outs = [], [], [], []
    for alloc in nc.m.functions[0].allocations:
        if not isinstance(alloc, mybir.MemoryLocationSet):
            continue
        name = alloc.memorylocations[0].name
        if alloc.kind == "ExternalInput":
            if name != partition_name:
                in_names.append(name)
        elif alloc.kind == "ExternalOutput":
            out_names.append(name)
            shape = tuple(alloc.tensor_shape)
            dtype = mybir.dt.np(alloc.dtype)
            out_avals.append(jax.core.ShapedArray(shape, dtype))
            zero_outs.append(_np.zeros(shape, dtype))
    n_params = len(in_names)
    n_outs = len(out_avals)
    all_in_names = in_names + out_names
    if partition_name is not None:
        all_in_names = all_in_names + [partition_name]
    donate = tuple(range(n_params, n_params + n_outs))

    def _body(*args):
        operands = list(args)
        if partition_name is not None:
            operands.append(bass2jax.partition_id_tensor())
        outs = bass2jax._bass_exec_p.bind(
            *operands,
            out_avals=tuple(out_avals),
            in_names=tuple(all_in_names),
            out_names=tuple(out_names),
            lowering_input_output_aliases=(),
            sim_require_finite=True,
            sim_require_nnan=True,
            nc=nc,
        )
        return tuple(outs)

    devices = jax.devices()[:N_CORES]
    mesh = Mesh(_np.asarray(devices), ("core",))
    sharded = jax.jit(
        shard_map(
            _body, mesh=mesh,
            in_specs=(PartitionSpec("core"),) * (n_params + n_outs),
            out_specs=(PartitionSpec("core"),) * n_outs,
            check_rep=False,
        ),
        donate_argnums=donate,
        keep_unused=True,
    )
    _CACHE["fn"] = (sharded, in_names, out_names, zero_outs)
    _CACHE["meta"] = (nc, out_avals, n_params, partition_name)
    _CACHE["all_in_names"] = all_in_names
    return _CACHE["fn"]


def run_cores_timed(in_maps, repeat=16, iters=3):
    """Measure per-exec time with device-resident inputs: queue `repeat`
    async executions and block once; per-exec = (T_repeat - T_1)/(repeat-1)
    cancels dispatch/RTT overhead that pipelines across queued execs."""
    import time
    import numpy as _np
    import jax
    from jax.sharding import Mesh, PartitionSpec, NamedSharding
    from jax.experimental.shard_map import shard_map
    from concourse import bass2jax

    _get_runner()
    nc, out_avals, n_params, partition_name = _CACHE["meta"]
    in_names = _CACHE["fn"][1]
    out_names = _CACHE["fn"][2]
    zero_outs = _CACHE["fn"][3]
    all_in_names = _CACHE["all_in_names"]

    def _body(*args):
        operands = list(args)
        if partition_name is not None:
            operands.append(bass2jax.partition_id_tensor())
        outs = bass2jax._bass_exec_p.bind(
            *operands,
            out_avals=tuple(out_avals),
            in_names=tuple(all_in_names),
            out_names=tuple(out_names),
            lowering_input_output_aliases=(),
            sim_require_finite=True,
            sim_require_nnan=True,
            nc=nc,
        )
        return tuple(outs)

    devices = jax.devices()[:N_CORES]
    mesh = Mesh(_np.asarray(devices), ("core",))
    n_outs = len(out_avals)
    fn = jax.jit(
        shard_map(
            _body, mesh=mesh,
            in_specs=(PartitionSpec("core"),) * (n_params + n_outs),
            out_specs=(PartitionSpec("core"),) * n_outs,
            check_rep=False,
        ),
        keep_unused=True,
    )
    sh = NamedSharding(mesh, PartitionSpec("core"))
    dev_in = [
        jax.device_put(
            _np.concatenate([_np.asarray(in_maps[c][n]) for c in range(N_CORES)],
                            axis=0), sh)
        for n in in_names
    ]
    dev_zero = [
        jax.device_put(
            _np.zeros((N_CORES * z.shape[0], *z.shape[1:]), z.dtype), sh)
        for z in zero_outs
    ]
    args = dev_in + dev_zero
    jax.block_until_ready(fn(*args))       # compile + warm
    t1s, tks = [], []
    for _ in range(iters):
        t0 = time.perf_counter()
        jax.block_until_ready(fn(*args))
        t1s.append(time.perf_counter() - t0)
    for _ in range(iters):
        t0 = time.perf_counter()
        outs = None
        for _i in range(repeat):
            outs = fn(*args)
        jax.block_until_ready(outs)
        tks.append(time.perf_counter() - t0)
    t1, tk = min(t1s), min(tks)
    per_exec = (tk - t1) / (repeat - 1)
    print(f"single-call: {t1*1e3:.2f} ms   {repeat}-queued: {tk*1e3:.2f} ms")
    return per_exec, (t1s, tks)


def run_cores(in_maps):
    """Run the SPMD kernel; in_maps is a list of 8 dicts name->array."""
    import numpy as _np
    sharded, in_names, out_names, zero_outs = _get_runner()
    concat_in = [
        _np.concatenate([_np.asarray(in_maps[c][n]) for c in range(N_CORES)], axis=0)
        for n in in_names
    ]
    concat_zeros = [
        _np.zeros((N_CORES * z.shape[0], *z.shape[1:]), z.dtype) for z in zero_outs
    ]
    out_arrs = sharded(*concat_in, *concat_zeros)
    per_core = []
    for c in range(N_CORES):
        d = {}
        for i, n in enumerate(out_names):
            full = _np.asarray(out_arrs[i])
            sh = full.shape[0] // N_CORES
            d[n] = full[c * sh:(c + 1) * sh]
        per_core.append(d)
    return per_core


def kernel(x, token_positions, W_qkv, W_out):
    x = np.asarray(x, dtype=np.float32)
    token_positions = np.asarray(token_positions)
    W_qkv = np.asarray(W_qkv, dtype=np.float32)
    W_out = np.asarray(W_out, dtype=np.float32)

    in_maps = [
        prep_core_inputs(x, token_positions, W_qkv, W_out, c)
        for c in range(N_CORES)
    ]
    res = run_cores(in_maps)
    b = x.shape[0]
    final = np.empty((b, SEQ, D_MODEL), dtype=np.float32)
    for bb in range(b):
        final[bb] = res[2 * bb]["out"] + res[2 * bb + 1]["out"]
    return final


# revision 3
# speedup vs baseline: 3.6581x; 3.6581x over previous
"""Multi-head causal self-attention with RoPE on 8 Trainium2 cores (v2).

Reference semantics (d_model=1024, 16 heads, d_h=64, rope theta 1e4):
    qkv = x @ W_qkv.T ; q,k = rope(q),rope(k)
    out = softmax(causal(q k^T / 8)) @ v ; return out @ W_out.T

Sharding: core c -> (batch b = c//2, head-group hg = c%2, 8 heads each).
Each core computes a partial output projection for its head group; the
host sums the two partials per batch. No on-device collectives.

v2 vs baseline:
  - bf16 operands everywhere (fp32 PSUM accumulation, fp32 output);
    halves DMA traffic and SBUF footprint, enables 2x DVE modes.
  - V stays resident in SBUF (no DRAM spill/reload).
  - PSUM evacuation copies, softmax normalize (partition_broadcast +
    divide) run on the idle GpSimd/Pool engine (proxy ucode library).
  - diagonal score tiles compute only the live column subrange
    (scores matmul, exp, PV all sliced; single [128,128] tri mask).
  - software-pipelined emission: attention for token-block nt (Act-heavy)
    is interleaved instruction-by-instruction with the projections for
    block nt+1 and the output projection of block nt-1 (PE-heavy), so
    the tensor engine never drains while exp runs.
"""

import numpy as np
import ml_dtypes

BF16 = ml_dtypes.bfloat16

D_MODEL = 1024
SEQ = 2048
N_HEADS = 16
D_H = 64
H_PER_CORE = 8
ROPE_THETA = 10000.0
N_CORES = 8

TQ = 512          # token block (free-dim tile)
NT = SEQ // TQ    # 4
NDC = D_MODEL // 128  # 8 contraction chunks


# ---------------------------------------------------------------- host math

def _a_perm():
    """A-layout row order for one 512-row head group (8 heads x 64 dims).

    chunk0: even dims of heads 0-3, chunk1: even dims of heads 4-7,
    chunk2: odd dims of heads 0-3,  chunk3: odd dims of heads 4-7.
    """
    idx = []
    for parity in (0, 1):
        for group in (0, 1):
            for h in range(4):
                for f in range(32):
                    idx.append((group * 4 + h) * 64 + 2 * f + parity)
    return np.array(idx, dtype=np.int64)


def _perm_mats():
    """[P_e0, P_e1, P_o0, P_o1] as [src, dst] 0/1 matrices.

    HC chunk c (heads 2c, 2c+1; rows [h: evens(32) odds(32)]) is
    P_e(c%2).T @ A_even(c//2) + P_o(c%2).T @ A_odd(c//2).
    """
    mats = np.zeros((4, 128, 128), np.float32)
    for cm in range(2):
        for d in range(128):
            hp, within = d // 64, d % 64
            parity, f = within // 32, within % 32
            s = (2 * cm + hp) * 32 + f
            mats[parity * 2 + cm, s, d] = 1.0
    return mats


def prep_core_inputs(x, token_positions, W_qkv, W_out, core):
    b, hg = core // 2, core % 2
    ap = _a_perm()

    Wq = W_qkv[hg * 512:(hg + 1) * 512]
    Wk = W_qkv[D_MODEL + hg * 512:D_MODEL + (hg + 1) * 512]
    Wv = W_qkv[2 * D_MODEL + hg * 512:2 * D_MODEL + (hg + 1) * 512]

    pos = token_positions.astype(np.float32)
    invf = 1.0 / (ROPE_THETA ** (np.arange(0, D_H, 2, dtype=np.float32) / D_H))
    ang = pos[None, :] * invf[np.arange(128) % 32, None]      # [128, SEQ]

    tri = (np.arange(128)[:, None] <= np.arange(128)[None, :])

    return {
        "xT": np.ascontiguousarray(x[b].T).astype(BF16),
        "wqkT": np.ascontiguousarray(
            np.concatenate([Wq[ap], Wk[ap]], axis=0).T).astype(BF16),
        "wvT": np.ascontiguousarray(Wv.T).astype(BF16),
        "woutT": np.ascontiguousarray(
            W_out[:, hg * 512:(hg + 1) * 512].T).astype(BF16),
        "cosA": np.ascontiguousarray(np.cos(ang)),
        "sinA": np.ascontiguousarray(np.sin(ang)),
        "tri": tri.astype(BF16),
        "perm": _perm_mats().astype(BF16),
        "ones": np.kron(np.eye(2, dtype=np.float32),
                        np.ones((1, 64), np.float32)),
    }


# ---------------------------------------------------------------- bass build

def build_bass(split_waits=True, wait_limit=1, loop_n=1):
    import concourse.bass as bass
    import concourse.mybir as mybir
    import concourse.tile as tile

    f32 = mybir.dt.float32
    bf16 = mybir.dt.bfloat16

    nc = bass.Bass("TRN2", target_bir_lowering=False, debug=False)
    # this walrus build cannot encode the raw-ISA RANGE_CLEAR emitted by
    # gpsimd.sem_clear in the kernel tail; NRT re-initializes semaphores per
    # execution, so replace it with a nop (verified by repeat-run checks).
    nc.gpsimd.sem_clear = lambda rng: nc.gpsimd.nop(hint="semclear_skip")

    class S:
        pass

    s = S()
    s.nc = nc
    s.f32, s.bf16 = f32, bf16
    s.f32r = mybir.dt.float32r
    s.EXP = mybir.ActivationFunctionType.Exp

    s.xT = nc.declare_dram_parameter("xT", [D_MODEL, SEQ], bf16, isOutput=False)
    s.wqkT = nc.declare_dram_parameter("wqkT", [D_MODEL, 1024], bf16, isOutput=False)
    s.wvT = nc.declare_dram_parameter("wvT", [D_MODEL, 512], bf16, isOutput=False)
    s.woutT = nc.declare_dram_parameter("woutT", [512, D_MODEL], bf16, isOutput=False)
    s.cosA = nc.declare_dram_parameter("cosA", [128, SEQ], f32, isOutput=False)
    s.sinA = nc.declare_dram_parameter("sinA", [128, SEQ], f32, isOutput=False)
    s.triP = nc.declare_dram_parameter("tri", [128, 128], bf16, isOutput=False)
    s.permP = nc.declare_dram_parameter("perm", [4, 128, 128], bf16, isOutput=False)
    s.onesP = nc.declare_dram_parameter(
        "ones", [2, 128], mybir.dt.float32r, isOutput=False)
    s.out = nc.declare_dram_parameter("out", [SEQ, D_MODEL], f32, isOutput=True)

    import contextlib
    with tile.TileContext(nc) as tc:
        s.tc = tc
        loop_cm = tc.For_i(0, loop_n) if loop_n > 1 else contextlib.nullcontext()
        with loop_cm:
            _build_pools_and_emit(s, tc)

    if split_waits:
        _split_sync_waits(nc, limit=wait_limit)
    return nc


def _build_pools_and_emit(s, tc):
        with (
            tc.tile_pool(name="persist", bufs=1) as s.p_per,
            tc.tile_pool(name="rt", bufs=4) as s.p_rt,
            tc.tile_pool(name="re", bufs=4) as s.p_re,
            tc.tile_pool(name="e", bufs=8) as s.p_e,
            tc.tile_pool(name="rep", bufs=2) as s.p_rep,
            tc.tile_pool(name="ob", bufs=2) as s.p_ob,
            tc.tile_pool(name="psA", bufs=2, space="PSUM") as s.ps_A,
            tc.tile_pool(name="psS", bufs=3, space="PSUM") as s.ps_S,
            tc.tile_pool(name="psU", bufs=2, space="PSUM") as s.ps_U,
            tc.tile_pool(name="psM", bufs=1, space="PSUM") as s.ps_M,
        ):
            _emit(s)


def _split_sync_waits(nc, limit=1):
    """walrus in this container rejects instructions with more than ~1 sync
    wait. Move excess waits onto preceding same-engine NOPs (engine streams
    execute in order, so the waits still complete before the instruction)."""
    import concourse.mybir as mybir
    n = 0
    for fn in nc.m.functions:
        for blk in fn.blocks:
            out = []
            for inst in blk.instructions:
                si = inst.sync_info
                waits = list(si.on_wait) if si is not None else []
                if len(waits) > limit:
                    for w in waits[:-limit]:
                        n += 1
                        nop = mybir.InstNoOp(
                            name=f"wsplit-{n}",
                            engine=inst.engine,
                            sync_info=mybir.SyncInfo(on_wait=[w], on_update=[]),
                        )
                        out.append(nop)
                    inst.sync_info = mybir.SyncInfo(
                        on_wait=waits[-limit:], on_update=list(si.on_update))
                out.append(inst)
            blk.instructions = out
    return n


# ------------------------------------------------------------- emission

def _alloc_persistent(s):
    p, bf16, f32 = s.p_per, s.bf16, s.f32
    # x^T chunks, per (dim-chunk, token-block)
    s.xt = [[p.tile([128, TQ], bf16, tag=f"xt{kc}_{b}", name=f"xt{kc}_{b}")
             for b in range(NT)] for kc in range(NDC)]
    # weights
    s.wqk = [p.tile([128, 1024], bf16, tag=f"wqk{kc}", name=f"wqk{kc}")
             for kc in range(NDC)]
    s.wv = [p.tile([128, 512], bf16, tag=f"wv{kc}", name=f"wv{kc}")
            for kc in range(NDC)]
    s.wo = [p.tile([128, 1024], bf16, tag=f"wo{kc}", name=f"wo{kc}")
            for kc in range(4)]
    # rope tables per block
    s.cos = [p.tile([128, TQ], f32, tag=f"cos{b}", name=f"cos{b}")
             for b in range(NT)]
    s.sin = [p.tile([128, TQ], f32, tag=f"sin{b}", name=f"sin{b}")
             for b in range(NT)]
    s.tri = p.tile([128, 128], bf16, tag="tri", name="tri")
    s.ones2 = p.tile([2, 128], s.f32r, tag="ones", name="ones")
    s.perm = [p.tile([128, 128], bf16, tag=f"pm{j}", name=f"pm{j}")
              for j in range(4)]
    # q/k head-contiguous chunks, per (chunk, token-block)
    s.q = [[p.tile([128, TQ], bf16, tag=f"q{c}_{b}", name=f"q{c}_{b}")
            for b in range(NT)] for c in range(4)]
    s.k = [[p.tile([128, TQ], bf16, tag=f"k{c}_{b}", name=f"k{c}_{b}")
            for b in range(NT)] for c in range(4)]
    # V with per-head ones column, per 128-token tile
    s.v = [p.tile([128, H_PER_CORE * 65], bf16, tag=f"v{t}", name=f"v{t}")
           for t in range(SEQ // 128)]
    # attention output chunks, per (chunk, token-block)
    s.ao = [[p.tile([128, TQ], bf16, tag=f"ao{c}_{b}", name=f"ao{c}_{b}")
             for b in range(NT)] for c in range(4)]


def _dma_block(s, b):
    """Prefetch x columns + rope tables for token block b."""
    nc = s.nc
    sl = slice(b * TQ, (b + 1) * TQ)
    for kc in range(NDC):
        nc.sync.dma_start(s.xt[kc][b][:], s.xT[kc * 128:(kc + 1) * 128, sl])
    nc.sync.dma_start(s.cos[b][:], s.cosA[:, sl])
    nc.sync.dma_start(s.sin[b][:], s.sinA[:, sl])


def _v_unit(s, b, tt):
    """V projection for 128-token tile tt (inside block b); yields per step."""
    nc = s.nc
    t0 = (tt % 4) * 128
    vp = s.ps_M.tile([128, 512], s.f32, tag="mps", name="vps")
    for kc in range(NDC):
        nc.tensor.matmul(
            vp[:], s.xt[kc][b][:, t0:t0 + 128], s.wv[kc][:],
            start=(kc == 0), stop=(kc == NDC - 1))
        yield
    nc.vector.tensor_copy(
        s.v[tt][:].rearrange("p (h d) -> p h d", d=65)[:, :, 0:64],
        vp[:].rearrange("p (h d) -> p h d", d=64))
    yield


def _qk_unit(s, half, pair, b):
    """Project A-chunk pair for one block, rope, permute to HC layout."""
    nc, f32, bf16 = s.nc, s.f32, s.bf16
    ce = half * 4 + pair
    co = half * 4 + 2 + pair
    pe = s.ps_A.tile([128, TQ], f32, tag="aps", name="pe")
    po = s.ps_A.tile([128, TQ], f32, tag="aps", name="po")
    for kc in range(NDC):
        nc.tensor.matmul(
            pe[:], s.wqk[kc][:, ce * 128:(ce + 1) * 128], s.xt[kc][b][:],
            start=(kc == 0), stop=(kc == NDC - 1))
        yield
    for kc in range(NDC):
        nc.tensor.matmul(
            po[:], s.wqk[kc][:, co * 128:(co + 1) * 128], s.xt[kc][b][:],
            start=(kc == 0), stop=(kc == NDC - 1))
        yield
    # rope: e' = e*cos - o*sin ; o' = e*sin + o*cos   (ops ordered so each
    # psum tile is released after its two reads)
    a = s.p_rt.tile([128, TQ], f32, tag="rt", name="rt_a")
    a2 = s.p_rt.tile([128, TQ], f32, tag="rt", name="rt_a2")
    bb = s.p_rt.tile([128, TQ], f32, tag="rt", name="rt_b")
    b2 = s.p_rt.tile([128, TQ], f32, tag="rt", name="rt_b2")
    re = s.p_re.tile([128, TQ], bf16, tag="re", name="re")
    ro = s.p_re.tile([128, TQ], bf16, tag="re", name="ro")
    nc.vector.tensor_mul(a[:], pe[:], s.cos[b][:])
    nc.vector.tensor_mul(a2[:], pe[:], s.sin[b][:])
    yield
    nc.vector.tensor_mul(bb[:], po[:], s.sin[b][:])
    nc.vector.tensor_mul(b2[:], po[:], s.cos[b][:])
    yield
    nc.vector.tensor_sub(re[:], a[:], bb[:])
    nc.vector.tensor_add(ro[:], a2[:], b2[:])
    yield
    hc_tiles = s.q if half == 0 else s.k
    for cc in (0, 1):
        c = 2 * pair + cc
        pp = s.ps_A.tile([128, TQ], f32, tag="aps", name="pp")
        nc.tensor.matmul(pp[:], s.perm[cc][:], re[:], start=True, stop=False)
        yield
        nc.tensor.matmul(pp[:], s.perm[2 + cc][:], ro[:], start=False, stop=True)
        yield
        nc.vector.tensor_copy(hc_tiles[c][b][:], pp[:])
        yield


def _out_unit(s, b, mt):
    """Output projection for 128-token tile mt of block b."""
    nc = s.nc
    t0 = (mt % 4) * 128
    ob = s.p_ob.tile([128, D_MODEL], s.f32, tag="ob", name="ob")
    for do in range(2):
        op = s.ps_M.tile([128, 512], s.f32, tag="mps", name="ops")
        for kc in range(4):
            nc.tensor.matmul(
                op[:], s.ao[kc][b][:, t0:t0 + 128],
                s.wo[kc][:, do * 512:(do + 1) * 512],
                start=(kc == 0), stop=(kc == 3))
            yield
        nc.vector.tensor_copy(ob[:, do * 512:(do + 1) * 512], op[:])
        yield
    nc.sync.dma_start(s.out[mt * 128:(mt + 1) * 128, :], ob[:])
    yield


def _attn_s(s, h, qt, kt):
    """Scores matmul for one (head, q-block, k-tile); exp comes a step later
    so the Activation engine never stalls on a just-issued matmul."""
    nc = s.nc
    hc, ro = h // 2, (h % 2) * 64
    j = kt - 4 * qt          # >= 0 on the 4 diagonal tiles
    lo = max(j, 0) * 128     # first live column within the q block
    kb, k0 = kt // 4, (kt % 4) * 128
    sp = s.ps_S.tile([128, TQ], s.f32, tag="sps", name="sps")
    nc.tensor.matmul(
        sp[:, lo:TQ],
        s.k[hc][kb][ro:ro + 64, k0:k0 + 128],
        s.q[hc][qt][ro:ro + 64, lo:TQ],
        start=True, stop=True)
    return (h, kt, lo, j, sp)


def _attn_exp(s, item):
    h, kt, lo, j, sp = item
    e = s.p_e.tile([128, TQ], s.bf16, tag="e", name="e")
    s.nc.scalar.activation(e[:, lo:TQ], sp[:, lo:TQ], s.EXP, scale=0.125)
    if j >= 0:
        s.nc.vector.tensor_mul(
            e[:, lo:lo + 128], e[:, lo:lo + 128], s.tri[:])
    return (h, kt, lo, j, e)


def _attn_pv(s, qt, item):
    """PV accumulation; triggers the pair normalize when a head retires."""
    h, kt, lo, j, e = item
    if kt == 0:
        s.u_ps[h] = s.ps_U.tile([65, TQ], s.f32, tag="ups", name=f"u{h % 2}")
    s.nc.tensor.matmul(
        s.u_ps[h][:, lo:TQ],
        s.v[kt][:, h * 65:(h + 1) * 65],
        e[:, lo:TQ],
        start=(kt == 0), stop=(j == 3))
    if j == 3:
        _attn_norm(s, h, qt)


def _attn_norm(s, h, qt):
    """ao[head dims] = U[0:64] * broadcast(1 / U[64]): DVE reciprocal
    (PSUM row -> SBUF), K=1 ones matmul broadcasts across partitions, DVE
    copy, DVE multiply.  (GPSIMD cannot touch PSUM on hw and this walrus
    build cannot encode the PartitionBroadcast ISA op.)"""
    nc = s.nc
    hc, ro = h // 2, (h % 2) * 64
    rec = s.p_rep.tile([1, TQ], s.f32r, tag="rec", name="rec")
    with nc.allow_low_precision(reason="fp32r is fp32-width"):
        nc.vector.reciprocal(rec[:], s.u_ps[h][64:65, :])
    rpp = s.ps_M.tile([64, TQ], s.f32, tag="mps", name="rpp")
    nc.tensor.matmul(rpp[:], s.ones2[0:1, 0:64], rec[:], start=True, stop=True)
    rep = s.p_rep.tile([64, TQ], s.f32, tag="rep", name="rep")
    nc.vector.tensor_copy(rep[:], rpp[:])
    nc.vector.tensor_mul(
        s.ao[hc][qt][ro:ro + 64, :], s.u_ps[h][0:64, :], rep[:])


def _emit(s):
    nc = s.nc
    _alloc_persistent(s)

    # ---- prologue
    # interleave x-block0 and wv chunks so the first V-proj matmuls can
    # start as soon as their own operands land (DMA engine is serial)
    sl0 = slice(0, TQ)
    for kc in range(NDC):
        nc.sync.dma_start(s.xt[kc][0][:], s.xT[kc * 128:(kc + 1) * 128, sl0])
        nc.scalar.dma_start(s.wv[kc][:], s.wvT[kc * 128:(kc + 1) * 128, :])
    nc.scalar.dma_start(s.cos[0][:], s.cosA[:, sl0])
    nc.scalar.dma_start(s.sin[0][:], s.sinA[:, sl0])
    for kc in range(NDC):
        nc.scalar.dma_start(s.wqk[kc][:], s.wqkT[kc * 128:(kc + 1) * 128, :])
    nc.sync.dma_start(s.tri[:], s.triP[:])
    nc.sync.dma_start(s.ones2[:], s.onesP[:])
    for j in range(4):
        nc.sync.dma_start(s.perm[j][:], s.permP[j])
    _dma_block(s, 1)
    for kc in range(4):
        nc.sync.dma_start(s.wo[kc][:], s.woutT[kc * 128:(kc + 1) * 128, :])
    for t in range(SEQ // 128):
        nc.vector.memset(
            s.v[t][:].rearrange("p (h d) -> p h d", d=65)[:, :, 64:65], 1.0)

    # block-0 projections (nothing to overlap with yet)
    for tt in range(4):
        for _ in _v_unit(s, 0, tt):
            pass
    for pair in range(2):
        for half in (0, 1):
            for _ in _qk_unit(s, half, pair, 0):
                pass

    # ---- steady-state pipeline over q blocks
    for qt in range(NT):
        if qt + 2 < NT:
            _dma_block(s, qt + 2)

        NV, NQK, NOUT = 9, 25, 11    # micro-steps per unit type
        units, n_micro = [], 0
        if qt == 0:
            units += [_v_unit(s, 1, tt) for tt in range(4, 8)]
            units += [_qk_unit(s, half, pair, 1)
                      for pair in range(2) for half in (0, 1)]
            n_micro = 4 * NV + 4 * NQK
        elif qt == 1:
            units += [_out_unit(s, 0, mt) for mt in range(0, 4)]
            units += [_v_unit(s, 2, tt) for tt in range(8, 12)]
            units += [_qk_unit(s, half, pair, 2)
                      for pair in range(2) for half in (0, 1)]
            n_micro = 4 * NOUT + 4 * NV + 4 * NQK
        elif qt == 2:
            units += [_out_unit(s, 1, mt) for mt in range(4, 8)]
            units += [_v_unit(s, 3, tt) for tt in range(12, 16)]
            n_micro = 4 * NOUT + 4 * NV
        else:
            units += [_out_unit(s, 2, mt) for mt in range(8, 12)]
            n_micro = 4 * NOUT

        # qt=3's q/k arrive per head-pair chunk, emitted just-in-time below
        late_qk = {}
        if qt == 3:
            late_qk = {0: [_qk_unit(s, 0, 0, 3), _qk_unit(s, 1, 0, 3)],
                       2: [_qk_unit(s, 0, 1, 3), _qk_unit(s, 1, 1, 3)]}

        def steps():
            for u in units:
                yield from u
        step_iter = steps()
        n_kt = 4 * (4 * qt + 4)      # kt steps across the 4 head pairs
        per_step, acc = n_micro / max(n_kt, 1), 0.0

        s.u_ps = {}
        s.norm_ready = []
        sq, pend = [], []

        def micro():
            nonlocal acc
            acc += per_step / 2.0
            while acc >= 1.0:
                acc -= 1.0
                try:
                    next(step_iter)
                except StopIteration:
                    acc = -1e9

        for hp in range(4):
            for u in late_qk.get(2 * hp, ()):   # just-in-time q/k for qt=3
                for _ in u:
                    pass
            h0, h1 = 2 * hp, 2 * hp + 1
            for kt in range(4 * qt + 4):
                while sq:                       # exps for the previous step
                    pend.append(_attn_exp(s, sq.pop(0)))
                sq.append(_attn_s(s, h0, qt, kt))
                if len(pend) > 2:
                    _attn_pv(s, qt, pend.pop(0))
                micro()
                sq.append(_attn_s(s, h1, qt, kt))
                if len(pend) > 2:
                    _attn_pv(s, qt, pend.pop(0))
                micro()
        while sq:                               # drain the pair pipeline
            pend.append(_attn_exp(s, sq.pop(0)))
        while pend:
            micro()
            _attn_pv(s, qt, pend.pop(0))
        for _ in step_iter:
            pass

    # ---- epilogue: last output projection
    for mt in range(12, 16):
        for _ in _out_unit(s, 3, mt):
            pass


# ---------------------------------------------------------------- execution

_CACHE = {}


def _get_runner():
    if "fn" in _CACHE:
        return _CACHE["fn"]
    import jax
    import numpy as _np
    from jax.sharding import Mesh, PartitionSpec
    from jax.experimental.shard_map import shard_map
    import concourse.mybir as mybir
    from concourse import bass2jax

    bass2jax.install_neuronx_cc_hook()
    nc = build_bass()

    partition_name = (
        nc.partition_id_tensor.name if nc.partition_id_tensor else None)
    in_names, out_names, out_avals, zero_outs = [], [], [], []
    for alloc in nc.m.functions[0].allocations:
        if not isinstance(alloc, mybir.MemoryLocationSet):
            continue
        name = alloc.memorylocations[0].name
        if alloc.kind == "ExternalInput":
            if name != partition_name:
                in_names.append(name)
        elif alloc.kind == "ExternalOutput":
            out_names.append(name)
            shape = tuple(alloc.tensor_shape)
            dtype = mybir.dt.np(alloc.dtype)
            out_avals.append(jax.core.ShapedArray(shape, dtype))
            zero_outs.append(_np.zeros(shape, dtype))
    n_params = len(in_names)
    n_outs = len(out_avals)
    all_in_names = in_names + out_names
    if partition_name is not None:
        all_in_names = all_in_names + [partition_name]
    donate = tuple(range(n_params, n_params + n_outs))

    def _body(*args):
        operands = list(args)
        if partition_name is not None:
            operands.append(bass2jax.partition_id_tensor())
        outs = bass2jax._bass_exec_p.bind(
            *operands,
            out_avals=tuple(out_avals),
            in_names=tuple(all_in_names),
            out_names=tuple(out_names),
            lowering_input_output_aliases=(),
            sim_require_finite=True,
            sim_require_nnan=True,
            nc=nc,
        )
        return tuple(outs)

    devices = jax.devices()[:N_CORES]
    mesh = Mesh(_np.asarray(devices), ("core",))
    sharded = jax.jit(
        shard_map(
            _body, mesh=mesh,
            in_specs=(PartitionSpec("core"),) * (n_params + n_outs),
            out_specs=(PartitionSpec("core"),) * n_outs,
            check_rep=False,
        ),
        donate_argnums=donate,
        keep_unused=True,
    )
    _CACHE["fn"] = (sharded, in_names, out_names, zero_outs)
    _CACHE["meta"] = (nc, out_avals, n_params, partition_name)
    _CACHE["all_in_names"] = all_in_names
    return _CACHE["fn"]


def run_cores_timed(in_maps, repeat=48, iters=6):
    """Estimate per-exec DEVICE time by comparing a NEFF whose body runs
    `repeat` times inside an on-device hardware loop (tc.For_i) against the
    single-shot NEFF: per-exec = (T_loop - T_single)/(repeat-1).  Both
    timings are one host dispatch each, so the ~0.6 ms/exec host+axon
    dispatch overhead (which swamps queue-based timing through this stack)
    cancels, leaving genuine device execution time per kernel run."""
    import time
    import numpy as _np
    import jax
    from jax.sharding import Mesh, PartitionSpec, NamedSharding
    from jax.experimental.shard_map import shard_map
    import concourse.mybir as mybir
    from concourse import bass2jax

    _get_runner()
    in_names = _CACHE["fn"][1]
    zero_outs = _CACHE["fn"][3]

    def _make_fn(nc):
        partition_name = (
            nc.partition_id_tensor.name if nc.partition_id_tensor else None)
        l_in, l_out, l_avals = [], [], []
        for alloc in nc.m.functions[0].allocations:
            if not isinstance(alloc, mybir.MemoryLocationSet):
                continue
            name = alloc.memorylocations[0].name
            if alloc.kind == "ExternalInput":
                if name != partition_name:
                    l_in.append(name)
            elif alloc.kind == "ExternalOutput":
                l_out.append(name)
                l_avals.append(jax.core.ShapedArray(
                    tuple(alloc.tensor_shape), mybir.dt.np(alloc.dtype)))
        all_in = l_in + l_out
        if partition_name is not None:
            all_in = all_in + [partition_name]

        def _body(*args):
            operands = list(args)
            if partition_name is not None:
                operands.append(bass2jax.partition_id_tensor())
            return tuple(bass2jax._bass_exec_p.bind(
                *operands, out_avals=tuple(l_avals),
                in_names=tuple(all_in), out_names=tuple(l_out),
                lowering_input_output_aliases=(),
                sim_require_finite=True, sim_require_nnan=True, nc=nc))

        devices = jax.devices()[:N_CORES]
        mesh = Mesh(_np.asarray(devices), ("core",))
        n_ops = len(l_in) + len(l_out)
        fn = jax.jit(shard_map(
            _body, mesh=mesh,
            in_specs=(PartitionSpec("core"),) * n_ops,
            out_specs=(PartitionSpec("core"),) * len(l_out),
            check_rep=False), keep_unused=True)
        return fn, l_in

    nc1 = _CACHE["meta"][0]
    nck = build_bass(loop_n=repeat)
    fn1, _ = _make_fn(nc1)
    fnk, _ = _make_fn(nck)

    devices = jax.devices()[:N_CORES]
    mesh = Mesh(_np.asarray(devices), ("core",))
    sh = NamedSharding(mesh, PartitionSpec("core"))
    dev_in = [
        jax.device_put(
            _np.concatenate([_np.asarray(in_maps[c][n]) for c in range(N_CORES)],
                            axis=0), sh)
        for n in in_names
    ]
    dev_zero = [
        jax.device_put(
            _np.zeros((N_CORES * z.shape[0], *z.shape[1:]), z.dtype), sh)
        for z in zero_outs
    ]
    args = dev_in + dev_zero
    jax.block_until_ready(fn1(*args))      # compile + warm
    jax.block_until_ready(fnk(*args))
    t1s, tks = [], []
    for _ in range(iters):
        t0 = time.perf_counter()
        jax.block_until_ready(fn1(*args))
        t1s.append(time.perf_counter() - t0)
        t0 = time.perf_counter()
        jax.block_until_ready(fnk(*args))
        tks.append(time.perf_counter() - t0)
    t1, tk = min(t1s), min(tks)
    per_exec = (tk - t1) / (repeat - 1)
    print(f"single-call: {t1*1e3:.2f} ms   {repeat}-loop call: {tk*1e3:.2f} ms")
    return per_exec, (t1s, tks)


def run_cores(in_maps):
    """Run the SPMD kernel; in_maps is a list of 8 dicts name->array."""
    import numpy as _np
    sharded, in_names, out_names, zero_outs = _get_runner()
    concat_in = [
        _np.concatenate([_np.asarray(in_maps[c][n]) for c in range(N_CORES)], axis=0)
        for n in in_names
    ]
    concat_zeros = [
        _np.zeros((N_CORES * z.shape[0], *z.shape[1:]), z.dtype) for z in zero_outs
    ]
    out_arrs = sharded(*concat_in, *concat_zeros)
    per_core = []
    for c in range(N_CORES):
        d = {}
        for i, n in enumerate(out_names):
            full = _np.asarray(out_arrs[i])
            sh = full.shape[0] // N_CORES
            d[n] = full[c * sh:(c + 1) * sh]
        per_core.append(d)
    return per_core


def kernel(x, token_positions, W_qkv, W_out):
    x = np.asarray(x, dtype=np.float32)
    token_positions = np.asarray(token_positions)
    W_qkv = np.asarray(W_qkv, dtype=np.float32)
    W_out = np.asarray(W_out, dtype=np.float32)

    in_maps = [
        prep_core_inputs(x, token_positions, W_qkv, W_out, c)
        for c in range(N_CORES)
    ]
    res = run_cores(in_maps)
    b = x.shape[0]
    final = np.empty((b, SEQ, D_MODEL), dtype=np.float32)
    for bb in range(b):
        final[bb] = res[2 * bb]["out"] + res[2 * bb + 1]["out"]
    return final


# revision 4
# speedup vs baseline: 3.6995x; 1.0113x over previous
"""Multi-head causal self-attention with RoPE on 8 Trainium2 cores (v2).

Reference semantics (d_model=1024, 16 heads, d_h=64, rope theta 1e4):
    qkv = x @ W_qkv.T ; q,k = rope(q),rope(k)
    out = softmax(causal(q k^T / 8)) @ v ; return out @ W_out.T

Sharding: core c -> (batch b = c//2, head-group hg = c%2, 8 heads each).
Each core computes a partial output projection for its head group; the
host sums the two partials per batch. No on-device collectives.

vs the original fp32 baseline:
  - bf16 operands everywhere (fp32 PSUM accumulation, fp32 output);
    halves DMA traffic and SBUF footprint, enables 2x DVE modes.
  - V stays resident in SBUF (no DRAM spill/reload).
  - diagonal score tiles compute only the live column subrange
    (scores matmul, exp, PV all sliced; single [128,128] tri mask).
  - software-pipelined emission: attention for token-block nt (Act-heavy)
    is interleaved instruction-by-instruction with the projections for
    block nt+1 and the output projection of block nt-1 (PE-heavy), so
    the tensor engine never drains while exp runs; scores issue a step
    ahead of exp, PV trails exp, normalize fires as heads retire.
  - hw constraints found the hard way: GPSIMD cannot touch PSUM, its ISA
    ops (partition_broadcast) don't encode on this walrus, pool tensor
    ops are slow on real silicon (masks live on DVE), and >1 sync wait
    per instruction is rejected (_split_sync_waits).
  - build_bass(loop_n=K) wraps the body in an on-device For_i loop;
    run_cores_timed uses it to measure genuine per-exec device time
    ((T_loopK - T_1)/(K-1)), cancelling the ~0.6 ms/exec host dispatch
    overhead of this axon path that swamps queue-based timing.
"""

import numpy as np
import ml_dtypes

BF16 = ml_dtypes.bfloat16

D_MODEL = 1024
SEQ = 2048
N_HEADS = 16
D_H = 64
H_PER_CORE = 8
ROPE_THETA = 10000.0
N_CORES = 8

TQ = 512          # token block (free-dim tile)
NT = SEQ // TQ    # 4
NDC = D_MODEL // 128  # 8 contraction chunks


# ---------------------------------------------------------------- host math

def _a_perm():
    """A-layout row order for one 512-row head group (8 heads x 64 dims).

    chunk0: even dims of heads 0-3, chunk1: even dims of heads 4-7,
    chunk2: odd dims of heads 0-3,  chunk3: odd dims of heads 4-7.
    """
    idx = []
    for parity in (0, 1):
        for group in (0, 1):
            for h in range(4):
                for f in range(32):
                    idx.append((group * 4 + h) * 64 + 2 * f + parity)
    return np.array(idx, dtype=np.int64)


def _perm_mats():
    """[P_e0, P_e1, P_o0, P_o1] as [src, dst] 0/1 matrices.

    HC chunk c (heads 2c, 2c+1; rows [h: evens(32) odds(32)]) is
    P_e(c%2).T @ A_even(c//2) + P_o(c%2).T @ A_odd(c//2).
    """
    mats = np.zeros((4, 128, 128), np.float32)
    for cm in range(2):
        for d in range(128):
            hp, within = d // 64, d % 64
            parity, f = within // 32, within % 32
            s = (2 * cm + hp) * 32 + f
            mats[parity * 2 + cm, s, d] = 1.0
    return mats


def prep_core_inputs(x, token_positions, W_qkv, W_out, core):
    b, hg = core // 2, core % 2
    ap = _a_perm()

    Wq = W_qkv[hg * 512:(hg + 1) * 512]
    Wk = W_qkv[D_MODEL + hg * 512:D_MODEL + (hg + 1) * 512]
    Wv = W_qkv[2 * D_MODEL + hg * 512:2 * D_MODEL + (hg + 1) * 512]

    pos = token_positions.astype(np.float32)
    invf = 1.0 / (ROPE_THETA ** (np.arange(0, D_H, 2, dtype=np.float32) / D_H))
    ang = pos[None, :] * invf[np.arange(128) % 32, None]      # [128, SEQ]

    tri = (np.arange(128)[:, None] <= np.arange(128)[None, :])

    return {
        "xT": np.ascontiguousarray(x[b].T).astype(BF16),
        "wqkT": np.ascontiguousarray(
            np.concatenate([Wq[ap], Wk[ap]], axis=0).T).astype(BF16),
        "wvT": np.ascontiguousarray(Wv.T).astype(BF16),
        "woutT": np.ascontiguousarray(
            W_out[:, hg * 512:(hg + 1) * 512].T).astype(BF16),
        "cosA": np.ascontiguousarray(np.cos(ang)),
        "sinA": np.ascontiguousarray(np.sin(ang)),
        "tri": tri.astype(BF16),
        "perm": _perm_mats().astype(BF16),
        "ones": np.kron(np.eye(2, dtype=np.float32),
                        np.ones((1, 64), np.float32)),
    }


# ---------------------------------------------------------------- bass build

def build_bass(split_waits=True, wait_limit=1, loop_n=1):
    import concourse.bass as bass
    import concourse.mybir as mybir
    import concourse.tile as tile

    f32 = mybir.dt.float32
    bf16 = mybir.dt.bfloat16

    nc = bass.Bass("TRN2", target_bir_lowering=False, debug=False)
    # this walrus build cannot encode the raw-ISA RANGE_CLEAR emitted by
    # gpsimd.sem_clear in the kernel tail; NRT re-initializes semaphores per
    # execution, so replace it with a nop (verified by repeat-run checks).
    nc.gpsimd.sem_clear = lambda rng: nc.gpsimd.nop(hint="semclear_skip")

    class S:
        pass

    s = S()
    s.nc = nc
    s.f32, s.bf16 = f32, bf16
    s.f32r = mybir.dt.float32r
    s.EXP = mybir.ActivationFunctionType.Exp

    s.xT = nc.declare_dram_parameter("xT", [D_MODEL, SEQ], bf16, isOutput=False)
    s.wqkT = nc.declare_dram_parameter("wqkT", [D_MODEL, 1024], bf16, isOutput=False)
    s.wvT = nc.declare_dram_parameter("wvT", [D_MODEL, 512], bf16, isOutput=False)
    s.woutT = nc.declare_dram_parameter("woutT", [512, D_MODEL], bf16, isOutput=False)
    s.cosA = nc.declare_dram_parameter("cosA", [128, SEQ], f32, isOutput=False)
    s.sinA = nc.declare_dram_parameter("sinA", [128, SEQ], f32, isOutput=False)
    s.triP = nc.declare_dram_parameter("tri", [128, 128], bf16, isOutput=False)
    s.permP = nc.declare_dram_parameter("perm", [4, 128, 128], bf16, isOutput=False)
    s.onesP = nc.declare_dram_parameter(
        "ones", [2, 128], mybir.dt.float32r, isOutput=False)
    s.out = nc.declare_dram_parameter("out", [SEQ, D_MODEL], f32, isOutput=True)

    import contextlib
    with tile.TileContext(nc) as tc:
        s.tc = tc
        loop_cm = tc.For_i(0, loop_n) if loop_n > 1 else contextlib.nullcontext()
        with loop_cm:
            _build_pools_and_emit(s, tc)

    if split_waits:
        _split_sync_waits(nc, limit=wait_limit)
    return nc


def _build_pools_and_emit(s, tc):
        with (
            tc.tile_pool(name="persist", bufs=1) as s.p_per,
            tc.tile_pool(name="rt", bufs=4) as s.p_rt,
            tc.tile_pool(name="re", bufs=4) as s.p_re,
            tc.tile_pool(name="e", bufs=8) as s.p_e,
            tc.tile_pool(name="rep", bufs=2) as s.p_rep,
            tc.tile_pool(name="ob", bufs=2) as s.p_ob,
            tc.tile_pool(name="psA", bufs=2, space="PSUM") as s.ps_A,
            tc.tile_pool(name="psS", bufs=3, space="PSUM") as s.ps_S,
            tc.tile_pool(name="psU", bufs=2, space="PSUM") as s.ps_U,
            tc.tile_pool(name="psM", bufs=1, space="PSUM") as s.ps_M,
        ):
            _emit(s)


def _split_sync_waits(nc, limit=1):
    """walrus in this container rejects instructions with more than ~1 sync
    wait. Move excess waits onto preceding same-engine NOPs (engine streams
    execute in order, so the waits still complete before the instruction)."""
    import concourse.mybir as mybir
    n = 0
    for fn in nc.m.functions:
        for blk in fn.blocks:
            out = []
            for inst in blk.instructions:
                si = inst.sync_info
                waits = list(si.on_wait) if si is not None else []
                if len(waits) > limit:
                    for w in waits[:-limit]:
                        n += 1
                        nop = mybir.InstNoOp(
                            name=f"wsplit-{n}",
                            engine=inst.engine,
                            sync_info=mybir.SyncInfo(on_wait=[w], on_update=[]),
                        )
                        out.append(nop)
                    inst.sync_info = mybir.SyncInfo(
                        on_wait=waits[-limit:], on_update=list(si.on_update))
                out.append(inst)
            blk.instructions = out
    return n


# ------------------------------------------------------------- emission

def _alloc_persistent(s):
    p, bf16, f32 = s.p_per, s.bf16, s.f32
    # x^T chunks, per (dim-chunk, token-block)
    s.xt = [[p.tile([128, TQ], bf16, tag=f"xt{kc}_{b}", name=f"xt{kc}_{b}")
             for b in range(NT)] for kc in range(NDC)]
    # weights
    s.wqk = [p.tile([128, 1024], bf16, tag=f"wqk{kc}", name=f"wqk{kc}")
             for kc in range(NDC)]
    s.wv = [p.tile([128, 512], bf16, tag=f"wv{kc}", name=f"wv{kc}")
            for kc in range(NDC)]
    s.wo = [p.tile([128, 1024], bf16, tag=f"wo{kc}", name=f"wo{kc}")
            for kc in range(4)]
    # rope tables per block
    s.cos = [p.tile([128, TQ], f32, tag=f"cos{b}", name=f"cos{b}")
             for b in range(NT)]
    s.sin = [p.tile([128, TQ], f32, tag=f"sin{b}", name=f"sin{b}")
             for b in range(NT)]
    s.tri = p.tile([128, 128], bf16, tag="tri", name="tri")
    s.ones2 = p.tile([2, 128], s.f32r, tag="ones", name="ones")
    s.perm = [p.tile([128, 128], bf16, tag=f"pm{j}", name=f"pm{j}")
              for j in range(4)]
    # q/k head-contiguous chunks, per (chunk, token-block)
    s.q = [[p.tile([128, TQ], bf16, tag=f"q{c}_{b}", name=f"q{c}_{b}")
            for b in range(NT)] for c in range(4)]
    s.k = [[p.tile([128, TQ], bf16, tag=f"k{c}_{b}", name=f"k{c}_{b}")
            for b in range(NT)] for c in range(4)]
    # V with per-head ones column, per 128-token tile
    s.v = [p.tile([128, H_PER_CORE * 65], bf16, tag=f"v{t}", name=f"v{t}")
           for t in range(SEQ // 128)]
    # attention output chunks, per (chunk, token-block)
    s.ao = [[p.tile([128, TQ], bf16, tag=f"ao{c}_{b}", name=f"ao{c}_{b}")
             for b in range(NT)] for c in range(4)]


def _dma_block(s, b):
    """Prefetch x columns + rope tables for token block b."""
    nc = s.nc
    sl = slice(b * TQ, (b + 1) * TQ)
    for kc in range(NDC):
        nc.sync.dma_start(s.xt[kc][b][:], s.xT[kc * 128:(kc + 1) * 128, sl])
    nc.sync.dma_start(s.cos[b][:], s.cosA[:, sl])
    nc.sync.dma_start(s.sin[b][:], s.sinA[:, sl])


def _v_unit(s, b, tt):
    """V projection for 128-token tile tt (inside block b); yields per step."""
    nc = s.nc
    t0 = (tt % 4) * 128
    vp = s.ps_M.tile([128, 512], s.f32, tag="mps", name="vps")
    for kc in range(NDC):
        nc.tensor.matmul(
            vp[:], s.xt[kc][b][:, t0:t0 + 128], s.wv[kc][:],
            start=(kc == 0), stop=(kc == NDC - 1))
        yield
    nc.vector.tensor_copy(
        s.v[tt][:].rearrange("p (h d) -> p h d", d=65)[:, :, 0:64],
        vp[:].rearrange("p (h d) -> p h d", d=64))
    yield


def _qk_unit(s, half, pair, b):
    """Project A-chunk pair for one block, rope, permute to HC layout."""
    nc, f32, bf16 = s.nc, s.f32, s.bf16
    ce = half * 4 + pair
    co = half * 4 + 2 + pair
    pe = s.ps_A.tile([128, TQ], f32, tag="aps", name="pe")
    po = s.ps_A.tile([128, TQ], f32, tag="aps", name="po")
    for kc in range(NDC):
        nc.tensor.matmul(
            pe[:], s.wqk[kc][:, ce * 128:(ce + 1) * 128], s.xt[kc][b][:],
            start=(kc == 0), stop=(kc == NDC - 1))
        yield
    for kc in range(NDC):
        nc.tensor.matmul(
            po[:], s.wqk[kc][:, co * 128:(co + 1) * 128], s.xt[kc][b][:],
            start=(kc == 0), stop=(kc == NDC - 1))
        yield
    # rope: e' = e*cos - o*sin ; o' = e*sin + o*cos   (ops ordered so each
    # psum tile is released after its two reads)
    a = s.p_rt.tile([128, TQ], f32, tag="rt", name="rt_a")
    a2 = s.p_rt.tile([128, TQ], f32, tag="rt", name="rt_a2")
    bb = s.p_rt.tile([128, TQ], f32, tag="rt", name="rt_b")
    b2 = s.p_rt.tile([128, TQ], f32, tag="rt", name="rt_b2")
    re = s.p_re.tile([128, TQ], bf16, tag="re", name="re")
    ro = s.p_re.tile([128, TQ], bf16, tag="re", name="ro")
    nc.vector.tensor_mul(a[:], pe[:], s.cos[b][:])
    nc.vector.tensor_mul(a2[:], pe[:], s.sin[b][:])
    yield
    nc.vector.tensor_mul(bb[:], po[:], s.sin[b][:])
    nc.vector.tensor_mul(b2[:], po[:], s.cos[b][:])
    yield
    nc.vector.tensor_sub(re[:], a[:], bb[:])
    nc.vector.tensor_add(ro[:], a2[:], b2[:])
    yield
    hc_tiles = s.q if half == 0 else s.k
    for cc in (0, 1):
        c = 2 * pair + cc
        pp = s.ps_A.tile([128, TQ], f32, tag="aps", name="pp")
        nc.tensor.matmul(pp[:], s.perm[cc][:], re[:], start=True, stop=False)
        yield
        nc.tensor.matmul(pp[:], s.perm[2 + cc][:], ro[:], start=False, stop=True)
        yield
        nc.vector.tensor_copy(hc_tiles[c][b][:], pp[:])
        yield


def _out_unit(s, b, mt):
    """Output projection for 128-token tile mt of block b."""
    nc = s.nc
    t0 = (mt % 4) * 128
    ob = s.p_ob.tile([128, D_MODEL], s.f32, tag="ob", name="ob")
    for do in range(2):
        op = s.ps_M.tile([128, 512], s.f32, tag="mps", name="ops")
        for kc in range(4):
            nc.tensor.matmul(
                op[:], s.ao[kc][b][:, t0:t0 + 128],
                s.wo[kc][:, do * 512:(do + 1) * 512],
                start=(kc == 0), stop=(kc == 3))
            yield
        nc.vector.tensor_copy(ob[:, do * 512:(do + 1) * 512], op[:])
        yield
    nc.sync.dma_start(s.out[mt * 128:(mt + 1) * 128, :], ob[:])
    yield


def _attn_s(s, h, qt, kt):
    """Scores matmul for one (head, q-block, k-tile); exp comes a step later
    so the Activation engine never stalls on a just-issued matmul."""
    nc = s.nc
    hc, ro = h // 2, (h % 2) * 64
    j = kt - 4 * qt          # >= 0 on the 4 diagonal tiles
    lo = max(j, 0) * 128     # first live column within the q block
    kb, k0 = kt // 4, (kt % 4) * 128
    sp = s.ps_S.tile([128, TQ], s.f32, tag="sps", name="sps")
    nc.tensor.matmul(
        sp[:, lo:TQ],
        s.k[hc][kb][ro:ro + 64, k0:k0 + 128],
        s.q[hc][qt][ro:ro + 64, lo:TQ],
        start=True, stop=True)
    return (h, kt, lo, j, sp)


def _attn_exp(s, item):
    h, kt, lo, j, sp = item
    e = s.p_e.tile([128, TQ], s.bf16, tag="e", name="e")
    s.nc.scalar.activation(e[:, lo:TQ], sp[:, lo:TQ], s.EXP, scale=0.125)
    if j >= 0:
        s.nc.vector.tensor_mul(
            e[:, lo:lo + 128], e[:, lo:lo + 128], s.tri[:])
    return (h, kt, lo, j, e)


def _attn_pv(s, qt, item):
    """PV accumulation; triggers the pair normalize when a head retires."""
    h, kt, lo, j, e = item
    if kt == 0:
        s.u_ps[h] = s.ps_U.tile([65, TQ], s.f32, tag="ups", name=f"u{h % 2}")
    s.nc.tensor.matmul(
        s.u_ps[h][:, lo:TQ],
        s.v[kt][:, h * 65:(h + 1) * 65],
        e[:, lo:TQ],
        start=(kt == 0), stop=(j == 3))
    if j == 3:
        _attn_norm(s, h, qt)


def _attn_norm(s, h, qt):
    """ao[head dims] = U[0:64] * broadcast(1 / U[64]): DVE reciprocal
    (PSUM row -> SBUF), K=1 ones matmul broadcasts across partitions, DVE
    copy, DVE multiply.  (GPSIMD cannot touch PSUM on hw and this walrus
    build cannot encode the PartitionBroadcast ISA op.)"""
    nc = s.nc
    hc, ro = h // 2, (h % 2) * 64
    rec = s.p_rep.tile([1, TQ], s.f32r, tag="rec", name="rec")
    with nc.allow_low_precision(reason="fp32r is fp32-width"):
        nc.vector.reciprocal(rec[:], s.u_ps[h][64:65, :])
    rpp = s.ps_M.tile([64, TQ], s.f32, tag="mps", name="rpp")
    nc.tensor.matmul(rpp[:], s.ones2[0:1, 0:64], rec[:], start=True, stop=True)
    rep = s.p_rep.tile([64, TQ], s.f32, tag="rep", name="rep")
    nc.vector.tensor_copy(rep[:], rpp[:])
    nc.vector.tensor_mul(
        s.ao[hc][qt][ro:ro + 64, :], s.u_ps[h][0:64, :], rep[:])


def _emit(s):
    nc = s.nc
    _alloc_persistent(s)

    # ---- prologue
    # interleave x-block0 and wv chunks so the first V-proj matmuls can
    # start as soon as their own operands land (DMA engine is serial)
    sl0 = slice(0, TQ)
    for kc in range(NDC):
        nc.sync.dma_start(s.xt[kc][0][:], s.xT[kc * 128:(kc + 1) * 128, sl0])
        nc.scalar.dma_start(s.wv[kc][:], s.wvT[kc * 128:(kc + 1) * 128, :])
    nc.scalar.dma_start(s.cos[0][:], s.cosA[:, sl0])
    nc.scalar.dma_start(s.sin[0][:], s.sinA[:, sl0])
    for kc in range(NDC):
        nc.scalar.dma_start(s.wqk[kc][:], s.wqkT[kc * 128:(kc + 1) * 128, :])
    nc.sync.dma_start(s.tri[:], s.triP[:])
    nc.sync.dma_start(s.ones2[:], s.onesP[:])
    for j in range(4):
        nc.sync.dma_start(s.perm[j][:], s.permP[j])
    _dma_block(s, 1)
    for kc in range(4):
        nc.sync.dma_start(s.wo[kc][:], s.woutT[kc * 128:(kc + 1) * 128, :])
    for t in range(SEQ // 128):
        nc.vector.memset(
            s.v[t][:].rearrange("p (h d) -> p h d", d=65)[:, :, 64:65], 1.0)

    # block-0 projections (nothing to overlap with yet)
    for tt in range(4):
        for _ in _v_unit(s, 0, tt):
            pass
    for pair in range(2):
        for half in (0, 1):
            for _ in _qk_unit(s, half, pair, 0):
                pass

    # ---- steady-state pipeline over q blocks
    for qt in range(NT):
        if qt + 2 < NT:
            _dma_block(s, qt + 2)

        NV, NQK, NOUT = 9, 25, 11    # micro-steps per unit type
        units, n_micro = [], 0
        if qt == 0:
            units += [_v_unit(s, 1, tt) for tt in range(4, 8)]
            units += [_qk_unit(s, half, pair, 1)
                      for pair in range(2) for half in (0, 1)]
            n_micro = 4 * NV + 4 * NQK
        elif qt == 1:
            units += [_out_unit(s, 0, mt) for mt in range(0, 4)]
            units += [_v_unit(s, 2, tt) for tt in range(8, 12)]
            units += [_qk_unit(s, half, pair, 2)
                      for pair in range(2) for half in (0, 1)]
            n_micro = 4 * NOUT + 4 * NV + 4 * NQK
        elif qt == 2:
            units += [_out_unit(s, 1, mt) for mt in range(4, 8)]
            units += [_v_unit(s, 3, tt) for tt in range(12, 16)]
            n_micro = 4 * NOUT + 4 * NV
        else:
            units += [_out_unit(s, 2, mt) for mt in range(8, 12)]
            n_micro = 4 * NOUT

        # qt=3's q/k arrive per head-pair chunk, emitted just-in-time below
        late_qk = {}
        if qt == 3:
            late_qk = {0: [_qk_unit(s, 0, 0, 3), _qk_unit(s, 1, 0, 3)],
                       2: [_qk_unit(s, 0, 1, 3), _qk_unit(s, 1, 1, 3)]}

        def steps():
            for u in units:
                yield from u
        step_iter = steps()
        n_kt = 4 * (4 * qt + 4)      # kt steps across the 4 head pairs
        per_step, acc = n_micro / max(n_kt, 1), 0.0

        s.u_ps = {}
        s.norm_ready = []
        sq, pend = [], []

        def micro():
            nonlocal acc
            acc += per_step / 2.0
            while acc >= 1.0:
                acc -= 1.0
                try:
                    next(step_iter)
                except StopIteration:
                    acc = -1e9

        for hp in range(4):
            for u in late_qk.get(2 * hp, ()):   # just-in-time q/k for qt=3
                for _ in u:
                    pass
            h0, h1 = 2 * hp, 2 * hp + 1
            for kt in range(4 * qt + 4):
                while sq:                       # exps for the previous step
                    pend.append(_attn_exp(s, sq.pop(0)))
                sq.append(_attn_s(s, h0, qt, kt))
                if len(pend) > 2:
                    _attn_pv(s, qt, pend.pop(0))
                micro()
                sq.append(_attn_s(s, h1, qt, kt))
                if len(pend) > 2:
                    _attn_pv(s, qt, pend.pop(0))
                micro()
        while sq:                               # drain the pair pipeline
            pend.append(_attn_exp(s, sq.pop(0)))
        while pend:
            micro()
            _attn_pv(s, qt, pend.pop(0))
        for _ in step_iter:
            pass

    # ---- epilogue: last output projection
    for mt in range(12, 16):
        for _ in _out_unit(s, 3, mt):
            pass


# ---------------------------------------------------------------- execution

_CACHE = {}


def _get_runner():
    if "fn" in _CACHE:
        return _CACHE["fn"]
    import jax
    import numpy as _np
    from jax.sharding import Mesh, PartitionSpec
    from jax.experimental.shard_map import shard_map
    import concourse.mybir as mybir
    from concourse import bass2jax

    bass2jax.install_neuronx_cc_hook()
    nc = build_bass()

    partition_name = (
        nc.partition_id_tensor.name if nc.partition_id_tensor else None)
    in_names, out_names, out_avals, zero_outs = [], [], [], []
    for alloc in nc.m.functions[0].allocations:
        if not isinstance(alloc, mybir.MemoryLocationSet):
            continue
        name = alloc.memorylocations[0].name
        if alloc.kind == "ExternalInput":
            if name != partition_name:
                in_names.append(name)
        elif alloc.kind == "ExternalOutput":
            out_names.append(name)
            shape = tuple(alloc.tensor_shape)
            dtype = mybir.dt.np(alloc.dtype)
            out_avals.append(jax.core.ShapedArray(shape, dtype))
            zero_outs.append(_np.zeros(shape, dtype))
    n_params = len(in_names)
    n_outs = len(out_avals)
    all_in_names = in_names + out_names
    if partition_name is not None:
        all_in_names = all_in_names + [partition_name]
    donate = tuple(range(n_params, n_params + n_outs))

    def _body(*args):
        operands = list(args)
        if partition_name is not None:
            operands.append(bass2jax.partition_id_tensor())
        outs = bass2jax._bass_exec_p.bind(
            *operands,
            out_avals=tuple(out_avals),
            in_names=tuple(all_in_names),
            out_names=tuple(out_names),
            lowering_input_output_aliases=(),
            sim_require_finite=True,
            sim_require_nnan=True,
            nc=nc,
        )
        return tuple(outs)

    devices = jax.devices()[:N_CORES]
    mesh = Mesh(_np.asarray(devices), ("core",))
    sharded = jax.jit(
        shard_map(
            _body, mesh=mesh,
            in_specs=(PartitionSpec("core"),) * (n_params + n_outs),
            out_specs=(PartitionSpec("core"),) * n_outs,
            check_rep=False,
        ),
        donate_argnums=donate,
        keep_unused=True,
    )
    _CACHE["fn"] = (sharded, in_names, out_names, zero_outs)
    _CACHE["meta"] = (nc, out_avals, n_params, partition_name)
    _CACHE["all_in_names"] = all_in_names
    return _CACHE["fn"]


def run_cores_timed(in_maps, repeat=48, iters=6):
    """Estimate per-exec DEVICE time by comparing a NEFF whose body runs
    `repeat` times inside an on-device hardware loop (tc.For_i) against the
    single-shot NEFF: per-exec = (T_loop - T_single)/(repeat-1).  Both
    timings are one host dispatch each, so the ~0.6 ms/exec host+axon
    dispatch overhead (which swamps queue-based timing through this stack)
    cancels, leaving genuine device execution time per kernel run."""
    import time
    import numpy as _np
    import jax
    from jax.sharding import Mesh, PartitionSpec, NamedSharding
    from jax.experimental.shard_map import shard_map
    import concourse.mybir as mybir
    from concourse import bass2jax

    _get_runner()
    in_names = _CACHE["fn"][1]
    zero_outs = _CACHE["fn"][3]

    def _make_fn(nc):
        partition_name = (
            nc.partition_id_tensor.name if nc.partition_id_tensor else None)
        l_in, l_out, l_avals = [], [], []
        for alloc in nc.m.functions[0].allocations:
            if not isinstance(alloc, mybir.MemoryLocationSet):
                continue
            name = alloc.memorylocations[0].name
            if alloc.kind == "ExternalInput":
                if name != partition_name:
                    l_in.append(name)
            elif alloc.kind == "ExternalOutput":
                l_out.append(name)
                l_avals.append(jax.core.ShapedArray(
                    tuple(alloc.tensor_shape), mybir.dt.np(alloc.dtype)))
        all_in = l_in + l_out
        if partition_name is not None:
            all_in = all_in + [partition_name]

        def _body(*args):
            operands = list(args)
            if partition_name is not None:
                operands.append(bass2jax.partition_id_tensor())
            return tuple(bass2jax._bass_exec_p.bind(
                *operands, out_avals=tuple(l_avals),
                in_names=tuple(all_in), out_names=tuple(l_out),
                lowering_input_output_aliases=(),
                sim_require_finite=True, sim_require_nnan=True, nc=nc))

        devices = jax.devices()[:N_CORES]
        mesh = Mesh(_np.asarray(devices), ("core",))
        n_ops = len(l_in) + len(l_out)
        fn = jax.jit(shard_map(
            _body, mesh=mesh,
            in_specs=(PartitionSpec("core"),) * n_ops,
            out_specs=(PartitionSpec("core"),) * len(l_out),
            check_rep=False), keep_unused=True)
        return fn, l_in

    nc1 = _CACHE["meta"][0]
    nck = build_bass(loop_n=repeat)
    fn1, _ = _make_fn(nc1)
    fnk, _ = _make_fn(nck)

    devices = jax.devices()[:N_CORES]
    mesh = Mesh(_np.asarray(devices), ("core",))
    sh = NamedSharding(mesh, PartitionSpec("core"))
    dev_in = [
        jax.device_put(
            _np.concatenate([_np.asarray(in_maps[c][n]) for c in range(N_CORES)],
                            axis=0), sh)
        for n in in_names
    ]
    dev_zero = [
        jax.device_put(
            _np.zeros((N_CORES * z.shape[0], *z.shape[1:]), z.dtype), sh)
        for z in zero_outs
    ]
    args = dev_in + dev_zero
    jax.block_until_ready(fn1(*args))      # compile + warm
    jax.block_until_ready(fnk(*args))
    t1s, tks = [], []
    for _ in range(iters):
        t0 = time.perf_counter()
        jax.block_until_ready(fn1(*args))
        t1s.append(time.perf_counter() - t0)
        t0 = time.perf_counter()
        jax.block_until_ready(fnk(*args))
        tks.append(time.perf_counter() - t0)
    t1, tk = min(t1s), min(tks)
    per_exec = (tk - t1) / (repeat - 1)
    print(f"single-call: {t1*1e3:.2f} ms   {repeat}-loop call: {tk*1e3:.2f} ms")
    return per_exec, (t1s, tks)


def run_cores(in_maps):
    """Run the SPMD kernel; in_maps is a list of 8 dicts name->array."""
    import numpy as _np
    sharded, in_names, out_names, zero_outs = _get_runner()
    concat_in = [
        _np.concatenate([_np.asarray(in_maps[c][n]) for c in range(N_CORES)], axis=0)
        for n in in_names
    ]
    concat_zeros = [
        _np.zeros((N_CORES * z.shape[0], *z.shape[1:]), z.dtype) for z in zero_outs
    ]
    out_arrs = sharded(*concat_in, *concat_zeros)
    per_core = []
    for c in range(N_CORES):
        d = {}
        for i, n in enumerate(out_names):
            full = _np.asarray(out_arrs[i])
            sh = full.shape[0] // N_CORES
            d[n] = full[c * sh:(c + 1) * sh]
        per_core.append(d)
    return per_core


def kernel(x, token_positions, W_qkv, W_out):
    x = np.asarray(x, dtype=np.float32)
    token_positions = np.asarray(token_positions)
    W_qkv = np.asarray(W_qkv, dtype=np.float32)
    W_out = np.asarray(W_out, dtype=np.float32)

    in_maps = [
        prep_core_inputs(x, token_positions, W_qkv, W_out, c)
        for c in range(N_CORES)
    ]
    res = run_cores(in_maps)
    b = x.shape[0]
    final = np.empty((b, SEQ, D_MODEL), dtype=np.float32)
    for bb in range(b):
        final[bb] = res[2 * bb]["out"] + res[2 * bb + 1]["out"]
    return final
